# revision 4
# baseline (speedup 1.0000x reference)
"""DeepseekV3 MLA attention kernel for 8 Trainium2 NeuronCores — v1.

Sharding: 2-way data-parallel over batch x 4-way tensor-parallel over heads.
Core c handles batch b = c // 4 and heads [4*(c%4) .. 4*(c%4)+4).

All matmul operands are bf16 (psum accumulation fp32). All intermediates
(qT, kv latent, kT, v) stay SBUF-resident — no DRAM scratch roundtrips.

Phase A (per 512-col s-block): fused q projection (LoRA A*B folded on host,
RoPE on rope rows) + kv latent, then k/v head projections from the
just-computed kv block.
Phase B: causal attention (scoresT layout, max-free softmax; denominators
accumulated on the Pool engine + one ones-matmul per head/q-tile) and this
head-group's slice of w_o. Host sums the 4 partial outputs per batch.
"""

from contextlib import ExitStack
from dataclasses import dataclass

import numpy as np

import concourse.bacc as bacc
import concourse.bass_isa as bass_isa
import concourse.mybir as mybir
import concourse.tile as tile

F32 = mybir.dt.float32
BF = mybir.dt.bfloat16
F8 = mybir.dt.float8e4
WS = 64.0  # host pre-scale on fp8 q weights (values ~0.0157 are denormal in e4m3)
QKS = 8.0  # fp8 storage scale for q/k vectors (folded out via the exp scale)


@dataclass(frozen=True)
class Cfg:
    S: int = 2048          # sequence length (per batch)
    HID: int = 2048        # hidden dim
    QLR: int = 1536        # q lora rank (host-side only)
    KVLR: int = 512        # kv lora rank
    NH_G: int = 4          # heads per core
    DN: int = 128          # nope dim
    DR: int = 64           # rope dim
    DV: int = 128          # v head dim
    S1T: int = 512         # phase-A s-block width
    QT: int = 512          # attention q-tile width

    @property
    def SCALE(self):
        return 1.0 / float(np.sqrt(self.DN + self.DR))


CFG = Cfg()


def build_nc(C: Cfg, reps: int = 1):
    nc = bacc.Bacc("TRN2", target_bir_lowering=False, debug=False, num_devices=8)
    P = 128
    HO = C.HID // P                 # 16 contraction chunks of hidden
    NS1 = C.S // C.S1T              # 4 s-blocks
    KVC = C.KVLR // P               # 4
    NPAIR = C.NH_G // 2             # 2
    NQT = C.S // C.QT               # 4
    NDIAG = C.QT // P               # 4
    NVS = C.S // P                  # 16
    NQN = C.NH_G                    # q nope row-blocks == heads
    NWQ = NQN + NPAIR               # 6 fused q column blocks of 128
    NOT = C.HID // 512              # 4 output column tiles

    # ---- kernel I/O (bf16 except the fp8 q path; host pre-packs) ----
    hT = nc.dram_tensor("hT", [C.HID, C.S], BF, kind="ExternalInput").ap()
    hT8 = nc.dram_tensor("hT8", [C.HID, C.S], F8, kind="ExternalInput").ap()
    # fused q weights packed [hi, t, ho, 128] so one DMA covers one col-block
    w_qf = nc.dram_tensor("w_qf", [P, NWQ, HO, P], F8, kind="ExternalInput").ap()
    w_kva = nc.dram_tensor("w_kva", [P, KVC, HO, P], BF, kind="ExternalInput").ap()
    w_kbn = nc.dram_tensor("w_kbn", [C.KVLR, C.NH_G * C.DN], BF, kind="ExternalInput").ap()
    w_kbr = nc.dram_tensor("w_kbr", [C.KVLR, C.NH_G * C.DR], BF, kind="ExternalInput").ap()
    w_vb = nc.dram_tensor("w_vb", [C.KVLR, C.NH_G * C.DV], BF, kind="ExternalInput").ap()
    w_ob = nc.dram_tensor("w_ob", [C.NH_G * C.DV, C.HID], BF, kind="ExternalInput").ap()
    cos2 = nc.dram_tensor("cos2", [P, C.S], BF, kind="ExternalInput").ap()
    ssin2 = nc.dram_tensor("ssin2", [P, C.S], BF, kind="ExternalInput").ap()
    dmask = nc.dram_tensor("dmask", [C.QT, C.QT], BF, kind="ExternalInput").ap()
    outp = nc.dram_tensor("outp", [C.S, C.HID], F32, kind="ExternalOutput").ap()

    with tile.TileContext(nc) as tc:
        for rep in range(reps):
            with ExitStack() as tctx:
                per_pool = tctx.enter_context(
                    tc.tile_pool(name=f"persist{rep}", bufs=1))
                cos_sb = per_pool.tile([P, C.S], BF)
                ssin_sb = per_pool.tile([P, C.S], BF)
                # q/k packed for fp8 DoubleRow scores: slot0 = nope dims 0:96
                # on partitions 0:96; slot1 = nope 96:128 on partitions 0:32
                # plus rope 0:64 on partitions 32:96. One matmul contracts
                # all 192 dims per key-block.
                qp_sb = per_pool.tile([P, C.NH_G, 2, C.S], F8)
                kp_sb = per_pool.tile([P, C.NH_G, 2, C.S], F8)
                kv_sb = per_pool.tile([P, KVC, C.S], BF)
                v_sb = per_pool.tile([P, NVS, C.NH_G * C.DV], BF)

                def rope_evict8(rp_pool, ps_nat, dst, s0, slen, scale):
                    """psum rope rows (head pair) -> roped, fp8, into the
                    slot1[32:96] lanes of the pair's two head tiles."""
                    tmp = rp_pool.tile([P, slen], BF, tag="rope_tmp")
                    nc.scalar.mul(tmp[:], ps_nat[:], scale)
                    qs = rp_pool.tile([P, slen], BF, tag="rope_qs")
                    for g in range(4):
                        nc.sync.dma_start(
                            out=qs[(g ^ 1) * 32:(g ^ 1) * 32 + 32, :],
                            in_=tmp[g * 32:(g + 1) * 32, :])
                    m1 = rp_pool.tile([P, slen], BF, tag="rope_m1")
                    nc.vector.tensor_mul(m1[:], tmp[:], cos_sb[:, s0:s0 + slen])
                    nc.vector.tensor_mul(qs[:], qs[:], ssin_sb[:, s0:s0 + slen])
                    rop = rp_pool.tile([P, slen], F8, tag="rope_o")
                    nc.vector.tensor_add(rop[:], m1[:], qs[:])
                    return rop

                def pack_evict(dst, h, stage, s0, slen, rop, h0, h1):
                    """DMA a staged fp8 nope tile + roped pair tile into the
                    packed q/k layout."""
                    if stage is not None:
                        nc.sync.dma_start(
                            out=dst[0:96, h, 0, s0:s0 + slen], in_=stage[0:96, :])
                        nc.sync.dma_start(
                            out=dst[0:32, h, 1, s0:s0 + slen], in_=stage[96:128, :])
                    if rop is not None:
                        nc.sync.dma_start(
                            out=dst[32:96, h0, 1, s0:s0 + slen], in_=rop[0:64, :])
                        nc.sync.dma_start(
                            out=dst[32:96, h1, 1, s0:s0 + slen], in_=rop[64:128, :])

                # ===== Phase A: q/kv latent + k/v head projections =====
                with ExitStack() as ctx:
                    wq_pool = ctx.enter_context(tc.tile_pool(name=f"wqf{rep}", bufs=1))
                    ht_pool = ctx.enter_context(tc.tile_pool(name=f"ht{rep}", bufs=3))
                    ev_pool = ctx.enter_context(tc.tile_pool(name=f"s1ev{rep}", bufs=4))
                    rp_pool = ctx.enter_context(tc.tile_pool(name=f"s1rp{rep}", bufs=3))
                    ps_pool = ctx.enter_context(
                        tc.tile_pool(name=f"s1ps{rep}", bufs=6, space="PSUM"))

                    wqf_sb = wq_pool.tile([P, NWQ, HO, P], F8)
                    wkva_sb = wq_pool.tile([P, KVC, HO, P], BF)
                    wkn_sb = wq_pool.tile([P, KVC, C.NH_G * C.DN], BF)
                    wkr_sb = wq_pool.tile([P, KVC, C.NH_G * C.DR], BF)
                    wv_sb = wq_pool.tile([P, KVC, C.NH_G * C.DV], BF)
                    # per col-block DMAs so the first matmuls start early
                    for t in range(NWQ):
                        nc.sync.dma_start(out=wqf_sb[:, t], in_=w_qf[:, t])
                    nc.sync.dma_start(out=cos_sb[:], in_=cos2)
                    nc.sync.dma_start(out=ssin_sb[:], in_=ssin2)
                    for cc in range(KVC):
                        nc.sync.dma_start(out=wkva_sb[:, cc], in_=w_kva[:, cc])
                    nc.sync.dma_start(
                        out=wkn_sb[:], in_=w_kbn.rearrange("(co ci) m -> ci co m", ci=P))
                    nc.sync.dma_start(
                        out=wkr_sb[:], in_=w_kbr.rearrange("(co ci) m -> ci co m", ci=P))
                    nc.sync.dma_start(
                        out=wv_sb[:], in_=w_vb.rearrange("(co ci) m -> ci co m", ci=P))

                    hT_r = hT.rearrange("(ho hi) s -> hi ho s", hi=P)
                    hT8_r = hT8.rearrange("(ho hi) s -> hi ho s", hi=P)

                    ht_tiles = {}

                    def load_ht(st):
                        s0 = st * C.S1T
                        ht_sb = ht_pool.tile([P, HO, C.S1T], BF)
                        ht8_sb = ht_pool.tile([P, HO, C.S1T], F8, tag="ht8")
                        for hg in range(4):
                            nc.scalar.dma_start(
                                out=ht8_sb[:, hg * 4:(hg + 1) * 4, :],
                                in_=hT8_r[:, hg * 4:(hg + 1) * 4, s0:s0 + C.S1T])
                        for hg in range(4):
                            nc.scalar.dma_start(
                                out=ht_sb[:, hg * 4:(hg + 1) * 4, :],
                                in_=hT_r[:, hg * 4:(hg + 1) * 4, s0:s0 + C.S1T])
                        ht_tiles[st] = (ht_sb, ht8_sb)

                    load_ht(0)
                    load_ht(1)
                    for st in range(NS1):
                        s0 = st * C.S1T
                        ht_sb, ht8_sb = ht_tiles.pop(st)

                        def accum(lhs_sb):
                            ps = ps_pool.tile([P, C.S1T], F32, tag="ps1")
                            for h in range(HO):
                                nc.tensor.matmul(
                                    ps[:], lhs_sb[:, h, :], ht_sb[:, h, :],
                                    start=(h == 0), stop=(h == HO - 1))
                            return ps

                        def accum8(lhs_sb):
                            # fp8 DoubleRow: one matmul contracts a pair of
                            # 128-row hidden chunks at 0.5 cycles/row
                            ps = ps_pool.tile([P, C.S1T], F32, tag="ps1")
                            for hp in range(HO // 2):
                                nc.tensor.matmul(
                                    ps[:], lhs_sb[:, 2 * hp:2 * hp + 2, :],
                                    ht8_sb[:, 2 * hp:2 * hp + 2, :],
                                    start=(hp == 0), stop=(hp == HO // 2 - 1),
                                    perf_mode=mybir.MatmulPerfMode.DoubleRow)
                            return ps

                        for t in range(NQN):
                            ps = accum8(wqf_sb[:, t])
                            stage = ev_pool.tile([P, C.S1T], F8, tag="qstage")
                            nc.scalar.mul(stage[:], ps[:], QKS / WS)
                            pack_evict(qp_sb, t, stage, s0, C.S1T, None, 0, 0)
                        for pr in range(NPAIR):
                            ps = accum8(wqf_sb[:, NQN + pr])
                            rop = rope_evict8(rp_pool, ps, qp_sb, s0, C.S1T,
                                              scale=QKS / WS)
                            pack_evict(qp_sb, 0, None, s0, C.S1T, rop,
                                       2 * pr, 2 * pr + 1)
                        for cc in range(KVC):
                            ps = accum(wkva_sb[:, cc])
                            nc.vector.tensor_copy(kv_sb[:, cc, s0:s0 + C.S1T], ps[:])

                        # k/v projections for this s-block (kv now resident)
                        for h in range(C.NH_G):
                            ps = ps_pool.tile([P, C.S1T], F32, tag="ps1")
                            for cc in range(KVC):
                                nc.tensor.matmul(
                                    ps[:], wkn_sb[:, cc, h * C.DN:(h + 1) * C.DN],
                                    kv_sb[:, cc, s0:s0 + C.S1T],
                                    start=(cc == 0), stop=(cc == KVC - 1))
                            stage = ev_pool.tile([P, C.S1T], F8, tag="kstage")
                            nc.scalar.mul(stage[:], ps[:], QKS)
                            pack_evict(kp_sb, h, stage, s0, C.S1T, None, 0, 0)
                        for pr in range(NPAIR):
                            ps = ps_pool.tile([P, C.S1T], F32, tag="ps1")
                            for cc in range(KVC):
                                nc.tensor.matmul(
                                    ps[:], wkr_sb[:, cc, pr * P:(pr + 1) * P],
                                    kv_sb[:, cc, s0:s0 + C.S1T],
                                    start=(cc == 0), stop=(cc == KVC - 1))
                            rop = rope_evict8(rp_pool, ps, kp_sb, s0, C.S1T,
                                              scale=QKS)
                            pack_evict(kp_sb, 0, None, s0, C.S1T, rop,
                                       2 * pr, 2 * pr + 1)
                        for ssub in range(C.S1T // P):
                            vs = (s0 + ssub * P) // P
                            ps = ps_pool.tile([P, C.NH_G * C.DV], F32, tag="ps1")
                            for cc in range(KVC):
                                nc.tensor.matmul(
                                    ps[:], kv_sb[:, cc, s0 + ssub * P:s0 + (ssub + 1) * P],
                                    wv_sb[:, cc, :],
                                    start=(cc == 0), stop=(cc == KVC - 1))
                            nc.scalar.copy(v_sb[:, vs, :], ps[:])
                        if st + 2 < NS1:
                            load_ht(st + 2)

                # ================= Phase B: attention + w_o =================
                with ExitStack() as ctx:
                    const_pool = ctx.enter_context(tc.tile_pool(name=f"ac{rep}", bufs=1))
                    e_pool = ctx.enter_context(tc.tile_pool(name=f"ae{rep}", bufs=8))
                    ep_pool = ctx.enter_context(tc.tile_pool(name=f"aep{rep}", bufs=3))
                    es_pool = ctx.enter_context(tc.tile_pool(name=f"aes{rep}", bufs=3))
                    d_pool = ctx.enter_context(tc.tile_pool(name=f"ad{rep}", bufs=3))
                    ao_pool = ctx.enter_context(tc.tile_pool(name=f"aao{rep}", bufs=2))
                    oev_pool = ctx.enter_context(tc.tile_pool(name=f"aoe{rep}", bufs=4))
                    ps_s = ctx.enter_context(
                        tc.tile_pool(name=f"apss{rep}", bufs=3, space="PSUM"))
                    ps_d = ctx.enter_context(
                        tc.tile_pool(name=f"apsd{rep}", bufs=1, space="PSUM"))
                    ps_o = ctx.enter_context(
                        tc.tile_pool(name=f"apso{rep}", bufs=2, space="PSUM"))
                    ps_w = ctx.enter_context(
                        tc.tile_pool(name=f"apsw{rep}", bufs=2, space="PSUM"))

                    dm_sb = const_pool.tile([P, NDIAG, C.QT], BF)
                    nc.sync.dma_start(
                        out=dm_sb[:], in_=dmask.rearrange("(j ki) q -> ki j q", ki=P))
                    wo_sb = const_pool.tile([P, C.NH_G, C.HID], BF)
                    nc.sync.dma_start(
                        out=wo_sb[:], in_=w_ob.rearrange("(h d) o -> d h o", d=P))
                    ones_sb = const_pool.tile([P, P], BF)
                    nc.vector.memset(ones_sb[:], 1.0)

                    def emit_wo(q0, ao_sb):
                        for qs in range(C.QT // P):
                            for ot in range(NOT):
                                psw = ps_w.tile([P, 512], F32, tag="psw")
                                for h in range(C.NH_G):
                                    nc.tensor.matmul(
                                        psw[:], ao_sb[:, h, qs * P:(qs + 1) * P],
                                        wo_sb[:, h, ot * 512:(ot + 1) * 512],
                                        start=(h == 0), stop=(h == C.NH_G - 1))
                                oev = oev_pool.tile([P, 512], F32)
                                nc.vector.tensor_copy(oev[:], psw[:])
                                nc.sync.dma_start(
                                    out=outp[q0 + qs * P:q0 + (qs + 1) * P,
                                             ot * 512:(ot + 1) * 512],
                                    in_=oev[:])

                    # deferred per-head softmax tail: runs while the NEXT
                    # head's score matmuls keep the PE busy, so the in-order
                    # PE queue never stalls on the exp->sum->recip chain.
                    tails = []

                    def flush_tail():
                        esum, pso, ao_ap = tails.pop(0)
                        esb = e_pool.tile([P, C.QT], BF, tag="esb")
                        nc.scalar.copy(esb[:], esum[:])
                        psd = ps_d.tile([P, C.QT], F32, tag="psd")
                        nc.tensor.matmul(psd[:], ones_sb[:], esb[:],
                                         start=True, stop=True)
                        rec = d_pool.tile([P, C.QT], F32, tag="rec")
                        nc.vector.reciprocal(rec[:], psd[:])
                        nc.vector.tensor_mul(ao_ap, pso[:], rec[:])

                    wo_pend = None
                    for qt in range(NQT):
                        q0 = qt * C.QT
                        nkt = (qt + 1) * C.QT // P
                        ao_sb = ao_pool.tile([P, C.NH_G, C.QT], BF, tag="ao")
                        for pr in range(NPAIR):
                            for hh in range(2):
                                h = pr * 2 + hh
                                esum = es_pool.tile([P, C.QT], F32, tag="esum")
                                pso = ps_o.tile([P, C.QT], F32, tag="pso")

                                def consume(e_prev, ktp):
                                    nc.tensor.matmul(
                                        pso[:],
                                        v_sb[:, ktp, h * C.DV:(h + 1) * C.DV],
                                        e_prev[:],
                                        start=(ktp == 0), stop=(ktp == nkt - 1))

                                pends = []
                                epair = None
                                for kt in range(nkt):
                                    k0 = kt * P
                                    pss = ps_s.tile([P, C.QT], F32, tag="pss")
                                    nc.tensor.matmul(
                                        pss[:], kp_sb[0:96, h, :, k0:k0 + P],
                                        qp_sb[0:96, h, :, q0:q0 + C.QT],
                                        start=True, stop=True,
                                        perf_mode=mybir.MatmulPerfMode.DoubleRow)
                                    e_sb = e_pool.tile([P, C.QT], BF, tag="e")
                                    nc.scalar.activation(
                                        e_sb[:], pss[:],
                                        mybir.ActivationFunctionType.Exp,
                                        scale=C.SCALE / (QKS * QKS))
                                    j = kt - qt * NDIAG
                                    if j >= 0:
                                        nc.vector.tensor_mul(
                                            e_sb[:], e_sb[:], dm_sb[:, j, :])
                                    # denominator: DVE pairs e tiles, Pool
                                    # accumulates the pairs into esum
                                    if epair is None and kt < nkt - 1:
                                        epair = e_sb
                                    else:
                                        if epair is not None:
                                            ep = ep_pool.tile([P, C.QT], F32, tag="ep")
                                            nc.vector.tensor_add(
                                                ep[:], epair[:], e_sb[:])
                                            epair = None
                                        else:
                                            ep = e_sb
                                        if kt <= 1:
                                            nc.gpsimd.tensor_copy(esum[:], ep[:])
                                        else:
                                            nc.gpsimd.tensor_add(
                                                esum[:], esum[:], ep[:])
                                    if kt == 2 and tails:
                                        flush_tail()
                                    if len(pends) >= 2:
                                        consume(*pends.pop(0))
                                    pends.append((e_sb, kt))
                                for p in pends:
                                    consume(*p)
                                tails.append((esum, pso, ao_sb[:, h, :]))
                                if h == 0 and wo_pend is not None:
                                    emit_wo(*wo_pend)
                                    wo_pend = None
                        wo_pend = (q0, ao_sb)
                    while tails:
                        flush_tail()
                    emit_wo(*wo_pend)

    nc.compile()
    return nc


def rope_tables(C: Cfg):
    """cos2/ssin2 [128, S]: two stacked 64-row blocks (head pairs share)."""
    inv = 1.0 / (10000.0 ** (np.arange(0, C.DR, 2, dtype=np.float64) / C.DR))
    freqs = np.arange(C.S, dtype=np.float64)[:, None] * inv[None, :]  # [S, 32]
    emb = np.concatenate([freqs, freqs], axis=1)  # [S, 64]
    cos = np.cos(emb).T.astype(np.float32)   # [64, S]
    sin = np.sin(emb).T.astype(np.float32)
    ssin = sin.copy()
    ssin[: C.DR // 2] = -ssin[: C.DR // 2]
    cos2 = np.concatenate([cos, cos], axis=0)     # [128, S]
    ssin2 = np.concatenate([ssin, ssin], axis=0)
    return np.ascontiguousarray(cos2), np.ascontiguousarray(ssin2)


def host_inputs(C: Cfg, inputs: dict, core: int):
    """Build the per-core input map from full inputs (bf16 + fp8 q path)."""
    bf16 = mybir.dt.np(BF)
    f8 = mybir.dt.np(F8)
    NH = inputs["w_q_nope"].shape[1] // C.DN
    groups = NH // C.NH_G
    b = core // groups
    g = core % groups
    hs = slice(g * C.NH_G, (g + 1) * C.NH_G)
    P = 128
    HO = C.HID // P
    NWQ = C.NH_G + C.NH_G // 2

    bf = lambda x: np.ascontiguousarray(np.asarray(x, dtype=np.float32)).astype(bf16)

    hT = bf(inputs["hidden_states"][b].T)
    w_q_a = np.asarray(inputs["w_q_a"], dtype=np.float32)
    w_qbn = np.asarray(inputs["w_q_nope"], dtype=np.float32).reshape(
        C.QLR, NH, C.DN)[:, hs].reshape(C.QLR, -1)
    w_qbr = np.asarray(inputs["w_q_rope"], dtype=np.float32).reshape(
        C.QLR, NH, C.DR)[:, hs].reshape(C.QLR, -1)
    w_qf_full = np.concatenate([w_q_a @ w_qbn, w_q_a @ w_qbr], axis=1)  # [HID, 768]
    # pack [hi, t, ho, c]: w[ho*128+hi, t*128+c]; fp8 with x64 pre-scale
    w_qf = np.clip(
        w_qf_full.reshape(HO, P, NWQ, P).transpose(1, 2, 0, 3) * WS,
        -240.0, 240.0).astype(f8)
    w_qf = np.ascontiguousarray(w_qf)
    w_kva_full = np.asarray(inputs["w_kv_a"], dtype=np.float32)  # [HID, KVLR]
    KVC = C.KVLR // P
    w_kva = bf(w_kva_full.reshape(HO, P, KVC, P).transpose(1, 2, 0, 3))
    w_kbn = bf(np.asarray(inputs["w_k_nope"], dtype=np.float32).reshape(
        C.KVLR, NH, C.DN)[:, hs].reshape(C.KVLR, -1))
    w_kbr = bf(np.asarray(inputs["w_k_rope"], dtype=np.float32).reshape(
        C.KVLR, NH, C.DR)[:, hs].reshape(C.KVLR, -1))
    w_vb = bf(np.asarray(inputs["w_v"], dtype=np.float32).reshape(
        C.KVLR, NH, C.DV)[:, hs].reshape(C.KVLR, -1))
    w_ob = bf(np.asarray(inputs["w_o"], dtype=np.float32).reshape(
        NH, C.DV, C.HID)[hs].reshape(-1, C.HID))
    cos2, ssin2 = rope_tables(C)
    cm = np.asarray(inputs["causal_mask"])[0, 0]
    dmask = np.ascontiguousarray(cm[-C.QT:, -C.QT:].T.astype(np.float32))
    hT8 = np.ascontiguousarray(
        np.clip(np.asarray(inputs["hidden_states"][b].T, dtype=np.float32),
                -240.0, 240.0).astype(f8))
    return {
        "hT": hT, "hT8": hT8, "w_qf": w_qf, "w_kva": w_kva,
        "w_kbn": w_kbn, "w_kbr": w_kbr, "w_vb": w_vb, "w_ob": w_ob,
        "cos2": cos2.astype(bf16), "ssin2": ssin2.astype(bf16),
        "dmask": dmask.astype(bf16),
    }


_NC_CACHE = {}


def kernel(**inputs) -> np.ndarray:
    from concourse.bass_utils import run_bass_kernel_spmd

    C = CFG
    if "nc" not in _NC_CACHE:
        _NC_CACHE["nc"] = build_nc(C)
    nc = _NC_CACHE["nc"]

    in_maps = [host_inputs(C, inputs, c) for c in range(8)]
    res = run_bass_kernel_spmd(nc, in_maps, core_ids=list(range(8)))

    B = inputs["hidden_states"].shape[0]
    groups = 8 // B
    out = np.zeros((B, C.S, C.HID), dtype=np.float32)
    for c in range(8):
        out[c // groups] += res.results[c]["outp"]
    return out


# revision 5
# speedup vs baseline: 1.0924x; 1.0924x over previous
"""DeepseekV3 MLA attention kernel for 8 Trainium2 NeuronCores.

Sharding: 2-way data-parallel over batch x 4-way tensor-parallel over heads.
Core c handles batch b = c // 4 and heads [4*(c%4) .. 4*(c%4)+4).

Precision: fp32 psum accumulation everywhere. The q path (hidden -> fused
LoRA A*B q projection) and the q·k score matmuls run as fp8e4 DoubleRow
(2x128 contraction per instruction, host pre-scales folded out via the exp
scale); the kv latent, k/v projections, attention·V and w_o stay bf16 since
v-path quantization error passes undamped to the output. All intermediates
(packed q/k, kv latent, v) stay SBUF-resident — no DRAM scratch roundtrips.

Phase A (per 512-col s-block): fused q projection (RoPE applied on rope
rows, results packed into the fp8 DoubleRow [96,2]-contraction layout via
partition-moving DMAs) + kv latent, then k/v head projections from the
just-computed kv block.
Phase B: causal attention in scoresT layout — one DoubleRow matmul per
128-key block, max-free softmax (exp on ACT, denominators pair-summed on
DVE, accumulated on Pool, reduced by a single ones-matmul whose tail is
deferred one head so the in-order PE queue never stalls) — then this head
group's slice of w_o, deferred one q-tile for the same reason. Host sums
the 4 partial outputs per batch.
"""

from contextlib import ExitStack
from dataclasses import dataclass

import numpy as np

import concourse.bacc as bacc
import concourse.bass_isa as bass_isa
import concourse.mybir as mybir
import concourse.tile as tile

F32 = mybir.dt.float32
BF = mybir.dt.bfloat16
F8 = mybir.dt.float8e4
WS = 64.0  # host pre-scale on fp8 q weights (values ~0.0157 are denormal in e4m3)
QKS = 8.0  # fp8 storage scale for q/k vectors (folded out via the exp scale)


@dataclass(frozen=True)
class Cfg:
    S: int = 2048          # sequence length (per batch)
    HID: int = 2048        # hidden dim
    QLR: int = 1536        # q lora rank (host-side only)
    KVLR: int = 512        # kv lora rank
    NH_G: int = 4          # heads per core
    DN: int = 128          # nope dim
    DR: int = 64           # rope dim
    DV: int = 128          # v head dim
    S1T: int = 512         # phase-A s-block width
    QT: int = 512          # attention q-tile width

    @property
    def SCALE(self):
        return 1.0 / float(np.sqrt(self.DN + self.DR))


CFG = Cfg()


def build_nc(C: Cfg, reps: int = 1):
    nc = bacc.Bacc("TRN2", target_bir_lowering=False, debug=False, num_devices=8)
    P = 128
    HO = C.HID // P                 # 16 contraction chunks of hidden
    NS1 = C.S // C.S1T              # 4 s-blocks
    KVC = C.KVLR // P               # 4
    NPAIR = C.NH_G // 2             # 2
    NQT = C.S // C.QT               # 4
    NDIAG = C.QT // P               # 4
    NVS = C.S // P                  # 16
    NQN = C.NH_G                    # q nope row-blocks == heads
    NWQ = NQN + NPAIR               # 6 fused q column blocks of 128
    NOT = C.HID // 512              # 4 output column tiles

    # ---- kernel I/O (bf16 except the fp8 q path; host pre-packs) ----
    hT = nc.dram_tensor("hT", [C.HID, C.S], BF, kind="ExternalInput").ap()
    hT8 = nc.dram_tensor("hT8", [C.HID, C.S], F8, kind="ExternalInput").ap()
    # fused q weights packed [hi, t, ho, 128] so one DMA covers one col-block
    w_qf = nc.dram_tensor("w_qf", [P, NWQ, HO, P], F8, kind="ExternalInput").ap()
    w_kva = nc.dram_tensor("w_kva", [P, KVC, HO, P], BF, kind="ExternalInput").ap()
    w_kbn = nc.dram_tensor("w_kbn", [C.KVLR, C.NH_G * C.DN], BF, kind="ExternalInput").ap()
    w_kbr = nc.dram_tensor("w_kbr", [C.KVLR, C.NH_G * C.DR], BF, kind="ExternalInput").ap()
    w_vb = nc.dram_tensor("w_vb", [C.KVLR, C.NH_G * C.DV], BF, kind="ExternalInput").ap()
    w_ob = nc.dram_tensor("w_ob", [C.NH_G * C.DV, C.HID], BF, kind="ExternalInput").ap()
    cos2 = nc.dram_tensor("cos2", [P, C.S], BF, kind="ExternalInput").ap()
    ssin2 = nc.dram_tensor("ssin2", [P, C.S], BF, kind="ExternalInput").ap()
    dmask = nc.dram_tensor("dmask", [C.QT, C.QT], BF, kind="ExternalInput").ap()
    outp = nc.dram_tensor("outp", [C.S, C.HID], F32, kind="ExternalOutput").ap()

    with tile.TileContext(nc) as tc:
        for rep in range(reps):
            with ExitStack() as tctx:
                per_pool = tctx.enter_context(
                    tc.tile_pool(name=f"persist{rep}", bufs=1))
                cos_sb = per_pool.tile([P, C.S], BF)
                ssin_sb = per_pool.tile([P, C.S], BF)
                # q/k packed for fp8 DoubleRow scores: slot0 = nope dims 0:96
                # on partitions 0:96; slot1 = nope 96:128 on partitions 0:32
                # plus rope 0:64 on partitions 32:96. One matmul contracts
                # all 192 dims per key-block.
                qp_sb = per_pool.tile([P, C.NH_G, 2, C.S], F8)
                kp_sb = per_pool.tile([P, C.NH_G, 2, C.S], F8)
                kv_sb = per_pool.tile([P, KVC, C.S], BF)
                v_sb = per_pool.tile([P, NVS, C.NH_G * C.DV], BF)

                def rope_evict8(rp_pool, ps_nat, dst, s0, slen, scale):
                    """psum rope rows (head pair) -> roped, fp8, into the
                    slot1[32:96] lanes of the pair's two head tiles."""
                    tmp = rp_pool.tile([P, slen], BF, tag="rope_tmp")
                    nc.scalar.mul(tmp[:], ps_nat[:], scale)
                    qs = rp_pool.tile([P, slen], BF, tag="rope_qs")
                    for g in range(4):
                        nc.sync.dma_start(
                            out=qs[(g ^ 1) * 32:(g ^ 1) * 32 + 32, :],
                            in_=tmp[g * 32:(g + 1) * 32, :])
                    m1 = rp_pool.tile([P, slen], BF, tag="rope_m1")
                    nc.vector.tensor_mul(m1[:], tmp[:], cos_sb[:, s0:s0 + slen])
                    nc.vector.tensor_mul(qs[:], qs[:], ssin_sb[:, s0:s0 + slen])
                    rop = rp_pool.tile([P, slen], F8, tag="rope_o")
                    nc.vector.tensor_add(rop[:], m1[:], qs[:])
                    return rop

                def pack_evict(dst, h, stage, s0, slen, rop, h0, h1):
                    """DMA a staged fp8 nope tile + roped pair tile into the
                    packed q/k layout."""
                    if stage is not None:
                        nc.sync.dma_start(
                            out=dst[0:96, h, 0, s0:s0 + slen], in_=stage[0:96, :])
                        nc.sync.dma_start(
                            out=dst[0:32, h, 1, s0:s0 + slen], in_=stage[96:128, :])
                    if rop is not None:
                        nc.sync.dma_start(
                            out=dst[32:96, h0, 1, s0:s0 + slen], in_=rop[0:64, :])
                        nc.sync.dma_start(
                            out=dst[32:96, h1, 1, s0:s0 + slen], in_=rop[64:128, :])

                # ===== Phase A: q/kv latent + k/v head projections =====
                with ExitStack() as ctx:
                    wq_pool = ctx.enter_context(tc.tile_pool(name=f"wqf{rep}", bufs=1))
                    ht_pool = ctx.enter_context(tc.tile_pool(name=f"ht{rep}", bufs=3))
                    ev_pool = ctx.enter_context(tc.tile_pool(name=f"s1ev{rep}", bufs=4))
                    rp_pool = ctx.enter_context(tc.tile_pool(name=f"s1rp{rep}", bufs=3))
                    ps_pool = ctx.enter_context(
                        tc.tile_pool(name=f"s1ps{rep}", bufs=6, space="PSUM"))

                    wqf_sb = wq_pool.tile([P, NWQ, HO, P], F8)
                    wkva_sb = wq_pool.tile([P, KVC, HO, P], BF)
                    wkn_sb = wq_pool.tile([P, KVC, C.NH_G * C.DN], BF)
                    wkr_sb = wq_pool.tile([P, KVC, C.NH_G * C.DR], BF)
                    wv_sb = wq_pool.tile([P, KVC, C.NH_G * C.DV], BF)
                    # per col-block DMAs so the first matmuls start early
                    for t in range(NWQ):
                        nc.sync.dma_start(out=wqf_sb[:, t], in_=w_qf[:, t])
                    nc.sync.dma_start(out=cos_sb[:], in_=cos2)
                    nc.sync.dma_start(out=ssin_sb[:], in_=ssin2)
                    for cc in range(KVC):
                        nc.sync.dma_start(out=wkva_sb[:, cc], in_=w_kva[:, cc])
                    nc.sync.dma_start(
                        out=wkn_sb[:], in_=w_kbn.rearrange("(co ci) m -> ci co m", ci=P))
                    nc.sync.dma_start(
                        out=wkr_sb[:], in_=w_kbr.rearrange("(co ci) m -> ci co m", ci=P))
                    nc.sync.dma_start(
                        out=wv_sb[:], in_=w_vb.rearrange("(co ci) m -> ci co m", ci=P))

                    hT_r = hT.rearrange("(ho hi) s -> hi ho s", hi=P)
                    hT8_r = hT8.rearrange("(ho hi) s -> hi ho s", hi=P)

                    ht_tiles = {}

                    def load_ht(st):
                        s0 = st * C.S1T
                        ht_sb = ht_pool.tile([P, HO, C.S1T], BF)
                        ht8_sb = ht_pool.tile([P, HO, C.S1T], F8, tag="ht8")
                        for hg in range(4):
                            nc.scalar.dma_start(
                                out=ht8_sb[:, hg * 4:(hg + 1) * 4, :],
                                in_=hT8_r[:, hg * 4:(hg + 1) * 4, s0:s0 + C.S1T])
                        for hg in range(4):
                            nc.scalar.dma_start(
                                out=ht_sb[:, hg * 4:(hg + 1) * 4, :],
                                in_=hT_r[:, hg * 4:(hg + 1) * 4, s0:s0 + C.S1T])
                        ht_tiles[st] = (ht_sb, ht8_sb)

                    load_ht(0)
                    load_ht(1)
                    for st in range(NS1):
                        s0 = st * C.S1T
                        ht_sb, ht8_sb = ht_tiles.pop(st)

                        def accum(lhs_sb):
                            ps = ps_pool.tile([P, C.S1T], F32, tag="ps1")
                            for h in range(HO):
                                nc.tensor.matmul(
                                    ps[:], lhs_sb[:, h, :], ht_sb[:, h, :],
                                    start=(h == 0), stop=(h == HO - 1))
                            return ps

                        def accum8(lhs_sb):
                            # fp8 DoubleRow: one matmul contracts a pair of
                            # 128-row hidden chunks at 0.5 cycles/row
                            ps = ps_pool.tile([P, C.S1T], F32, tag="ps1")
                            for hp in range(HO // 2):
                                nc.tensor.matmul(
                                    ps[:], lhs_sb[:, 2 * hp:2 * hp + 2, :],
                                    ht8_sb[:, 2 * hp:2 * hp + 2, :],
                                    start=(hp == 0), stop=(hp == HO // 2 - 1),
                                    perf_mode=mybir.MatmulPerfMode.DoubleRow)
                            return ps

                        for t in range(NQN):
                            ps = accum8(wqf_sb[:, t])
                            stage = ev_pool.tile([P, C.S1T], F8, tag="qstage")
                            nc.scalar.mul(stage[:], ps[:], QKS / WS)
                            pack_evict(qp_sb, t, stage, s0, C.S1T, None, 0, 0)
                        for pr in range(NPAIR):
                            ps = accum8(wqf_sb[:, NQN + pr])
                            rop = rope_evict8(rp_pool, ps, qp_sb, s0, C.S1T,
                                              scale=QKS / WS)
                            pack_evict(qp_sb, 0, None, s0, C.S1T, rop,
                                       2 * pr, 2 * pr + 1)
                        for cc in range(KVC):
                            ps = accum(wkva_sb[:, cc])
                            nc.vector.tensor_copy(kv_sb[:, cc, s0:s0 + C.S1T], ps[:])

                        # k/v projections for this s-block (kv now resident)
                        for h in range(C.NH_G):
                            ps = ps_pool.tile([P, C.S1T], F32, tag="ps1")
                            for cc in range(KVC):
                                nc.tensor.matmul(
                                    ps[:], wkn_sb[:, cc, h * C.DN:(h + 1) * C.DN],
                                    kv_sb[:, cc, s0:s0 + C.S1T],
                                    start=(cc == 0), stop=(cc == KVC - 1))
                            stage = ev_pool.tile([P, C.S1T], F8, tag="kstage")
                            nc.scalar.mul(stage[:], ps[:], QKS)
                            pack_evict(kp_sb, h, stage, s0, C.S1T, None, 0, 0)
                        for pr in range(NPAIR):
                            ps = ps_pool.tile([P, C.S1T], F32, tag="ps1")
                            for cc in range(KVC):
                                nc.tensor.matmul(
                                    ps[:], wkr_sb[:, cc, pr * P:(pr + 1) * P],
                                    kv_sb[:, cc, s0:s0 + C.S1T],
                                    start=(cc == 0), stop=(cc == KVC - 1))
                            rop = rope_evict8(rp_pool, ps, kp_sb, s0, C.S1T,
                                              scale=QKS)
                            pack_evict(kp_sb, 0, None, s0, C.S1T, rop,
                                       2 * pr, 2 * pr + 1)
                        for ssub in range(C.S1T // P):
                            vs = (s0 + ssub * P) // P
                            ps = ps_pool.tile([P, C.NH_G * C.DV], F32, tag="ps1")
                            for cc in range(KVC):
                                nc.tensor.matmul(
                                    ps[:], kv_sb[:, cc, s0 + ssub * P:s0 + (ssub + 1) * P],
                                    wv_sb[:, cc, :],
                                    start=(cc == 0), stop=(cc == KVC - 1))
                            nc.scalar.copy(v_sb[:, vs, :], ps[:])
                        if st + 2 < NS1:
                            load_ht(st + 2)

                # ================= Phase B: attention + w_o =================
                with ExitStack() as ctx:
                    const_pool = ctx.enter_context(tc.tile_pool(name=f"ac{rep}", bufs=1))
                    e_pool = ctx.enter_context(tc.tile_pool(name=f"ae{rep}", bufs=8))
                    ep_pool = ctx.enter_context(tc.tile_pool(name=f"aep{rep}", bufs=3))
                    es_pool = ctx.enter_context(tc.tile_pool(name=f"aes{rep}", bufs=3))
                    d_pool = ctx.enter_context(tc.tile_pool(name=f"ad{rep}", bufs=3))
                    ao_pool = ctx.enter_context(tc.tile_pool(name=f"aao{rep}", bufs=2))
                    oev_pool = ctx.enter_context(tc.tile_pool(name=f"aoe{rep}", bufs=4))
                    ps_s = ctx.enter_context(
                        tc.tile_pool(name=f"apss{rep}", bufs=3, space="PSUM"))
                    ps_d = ctx.enter_context(
                        tc.tile_pool(name=f"apsd{rep}", bufs=1, space="PSUM"))
                    ps_o = ctx.enter_context(
                        tc.tile_pool(name=f"apso{rep}", bufs=2, space="PSUM"))
                    ps_w = ctx.enter_context(
                        tc.tile_pool(name=f"apsw{rep}", bufs=2, space="PSUM"))

                    dm_sb = const_pool.tile([P, NDIAG, C.QT], BF)
                    nc.sync.dma_start(
                        out=dm_sb[:], in_=dmask.rearrange("(j ki) q -> ki j q", ki=P))
                    wo_sb = const_pool.tile([P, C.NH_G, C.HID], BF)
                    nc.sync.dma_start(
                        out=wo_sb[:], in_=w_ob.rearrange("(h d) o -> d h o", d=P))
                    ones_sb = const_pool.tile([P, P], BF)
                    nc.vector.memset(ones_sb[:], 1.0)

                    def emit_wo(q0, ao_sb):
                        for qs in range(C.QT // P):
                            for ot in range(NOT):
                                psw = ps_w.tile([P, 512], F32, tag="psw")
                                for h in range(C.NH_G):
                                    nc.tensor.matmul(
                                        psw[:], ao_sb[:, h, qs * P:(qs + 1) * P],
                                        wo_sb[:, h, ot * 512:(ot + 1) * 512],
                                        start=(h == 0), stop=(h == C.NH_G - 1))
                                oev = oev_pool.tile([P, 512], F32)
                                nc.vector.tensor_copy(oev[:], psw[:])
                                nc.sync.dma_start(
                                    out=outp[q0 + qs * P:q0 + (qs + 1) * P,
                                             ot * 512:(ot + 1) * 512],
                                    in_=oev[:])

                    # deferred per-head softmax tail: runs while the NEXT
                    # head's score matmuls keep the PE busy, so the in-order
                    # PE queue never stalls on the exp->sum->recip chain.
                    tails = []

                    def flush_tail():
                        esum, pso, ao_ap = tails.pop(0)
                        esb = e_pool.tile([P, C.QT], BF, tag="esb")
                        nc.scalar.copy(esb[:], esum[:])
                        psd = ps_d.tile([P, C.QT], F32, tag="psd")
                        nc.tensor.matmul(psd[:], ones_sb[:], esb[:],
                                         start=True, stop=True)
                        rec = d_pool.tile([P, C.QT], F32, tag="rec")
                        nc.vector.reciprocal(rec[:], psd[:])
                        nc.vector.tensor_mul(ao_ap, pso[:], rec[:])

                    wo_pend = None
                    for qt in range(NQT):
                        q0 = qt * C.QT
                        nkt = (qt + 1) * C.QT // P
                        ao_sb = ao_pool.tile([P, C.NH_G, C.QT], BF, tag="ao")
                        for pr in range(NPAIR):
                            for hh in range(2):
                                h = pr * 2 + hh
                                esum = es_pool.tile([P, C.QT], F32, tag="esum")
                                pso = ps_o.tile([P, C.QT], F32, tag="pso")

                                def consume(e_prev, ktp):
                                    nc.tensor.matmul(
                                        pso[:],
                                        v_sb[:, ktp, h * C.DV:(h + 1) * C.DV],
                                        e_prev[:],
                                        start=(ktp == 0), stop=(ktp == nkt - 1))

                                pends = []
                                epair = None
                                for kt in range(nkt):
                                    k0 = kt * P
                                    pss = ps_s.tile([P, C.QT], F32, tag="pss")
                                    nc.tensor.matmul(
                                        pss[:], kp_sb[0:96, h, :, k0:k0 + P],
                                        qp_sb[0:96, h, :, q0:q0 + C.QT],
                                        start=True, stop=True,
                                        perf_mode=mybir.MatmulPerfMode.DoubleRow)
                                    e_sb = e_pool.tile([P, C.QT], BF, tag="e")
                                    nc.scalar.activation(
                                        e_sb[:], pss[:],
                                        mybir.ActivationFunctionType.Exp,
                                        scale=C.SCALE / (QKS * QKS))
                                    j = kt - qt * NDIAG
                                    if j >= 0:
                                        nc.vector.tensor_mul(
                                            e_sb[:], e_sb[:], dm_sb[:, j, :])
                                    # denominator: DVE pairs e tiles, Pool
                                    # accumulates the pairs into esum
                                    if epair is None and kt < nkt - 1:
                                        epair = e_sb
                                    else:
                                        if epair is not None:
                                            ep = ep_pool.tile([P, C.QT], F32, tag="ep")
                                            nc.vector.tensor_add(
                                                ep[:], epair[:], e_sb[:])
                                            epair = None
                                        else:
                                            ep = e_sb
                                        if kt <= 1:
                                            nc.gpsimd.tensor_copy(esum[:], ep[:])
                                        else:
                                            nc.gpsimd.tensor_add(
                                                esum[:], esum[:], ep[:])
                                    if kt == 2 and tails:
                                        flush_tail()
                                    if len(pends) >= 2:
                                        consume(*pends.pop(0))
                                    pends.append((e_sb, kt))
                                for p in pends:
                                    consume(*p)
                                tails.append((esum, pso, ao_sb[:, h, :]))
                                if h == 0 and wo_pend is not None:
                                    emit_wo(*wo_pend)
                                    wo_pend = None
                        wo_pend = (q0, ao_sb)
                    while tails:
                        flush_tail()
                    emit_wo(*wo_pend)

    nc.compile()
    return nc


def rope_tables(C: Cfg):
    """cos2/ssin2 [128, S]: two stacked 64-row blocks (head pairs share)."""
    inv = 1.0 / (10000.0 ** (np.arange(0, C.DR, 2, dtype=np.float64) / C.DR))
    freqs = np.arange(C.S, dtype=np.float64)[:, None] * inv[None, :]  # [S, 32]
    emb = np.concatenate([freqs, freqs], axis=1)  # [S, 64]
    cos = np.cos(emb).T.astype(np.float32)   # [64, S]
    sin = np.sin(emb).T.astype(np.float32)
    ssin = sin.copy()
    ssin[: C.DR // 2] = -ssin[: C.DR // 2]
    cos2 = np.concatenate([cos, cos], axis=0)     # [128, S]
    ssin2 = np.concatenate([ssin, ssin], axis=0)
    return np.ascontiguousarray(cos2), np.ascontiguousarray(ssin2)


def host_inputs(C: Cfg, inputs: dict, core: int):
    """Build the per-core input map from full inputs (bf16 + fp8 q path)."""
    bf16 = mybir.dt.np(BF)
    f8 = mybir.dt.np(F8)
    NH = inputs["w_q_nope"].shape[1] // C.DN
    groups = NH // C.NH_G
    b = core // groups
    g = core % groups
    hs = slice(g * C.NH_G, (g + 1) * C.NH_G)
    P = 128
    HO = C.HID // P
    NWQ = C.NH_G + C.NH_G // 2

    bf = lambda x: np.ascontiguousarray(np.asarray(x, dtype=np.float32)).astype(bf16)

    hT = bf(inputs["hidden_states"][b].T)
    w_q_a = np.asarray(inputs["w_q_a"], dtype=np.float32)
    w_qbn = np.asarray(inputs["w_q_nope"], dtype=np.float32).reshape(
        C.QLR, NH, C.DN)[:, hs].reshape(C.QLR, -1)
    w_qbr = np.asarray(inputs["w_q_rope"], dtype=np.float32).reshape(
        C.QLR, NH, C.DR)[:, hs].reshape(C.QLR, -1)
    w_qf_full = np.concatenate([w_q_a @ w_qbn, w_q_a @ w_qbr], axis=1)  # [HID, 768]
    # pack [hi, t, ho, c]: w[ho*128+hi, t*128+c]; fp8 with x64 pre-scale
    w_qf = np.clip(
        w_qf_full.reshape(HO, P, NWQ, P).transpose(1, 2, 0, 3) * WS,
        -240.0, 240.0).astype(f8)
    w_qf = np.ascontiguousarray(w_qf)
    w_kva_full = np.asarray(inputs["w_kv_a"], dtype=np.float32)  # [HID, KVLR]
    KVC = C.KVLR // P
    w_kva = bf(w_kva_full.reshape(HO, P, KVC, P).transpose(1, 2, 0, 3))
    w_kbn = bf(np.asarray(inputs["w_k_nope"], dtype=np.float32).reshape(
        C.KVLR, NH, C.DN)[:, hs].reshape(C.KVLR, -1))
    w_kbr = bf(np.asarray(inputs["w_k_rope"], dtype=np.float32).reshape(
        C.KVLR, NH, C.DR)[:, hs].reshape(C.KVLR, -1))
    w_vb = bf(np.asarray(inputs["w_v"], dtype=np.float32).reshape(
        C.KVLR, NH, C.DV)[:, hs].reshape(C.KVLR, -1))
    w_ob = bf(np.asarray(inputs["w_o"], dtype=np.float32).reshape(
        NH, C.DV, C.HID)[hs].reshape(-1, C.HID))
    cos2, ssin2 = rope_tables(C)
    cm = np.asarray(inputs["causal_mask"])[0, 0]
    dmask = np.ascontiguousarray(cm[-C.QT:, -C.QT:].T.astype(np.float32))
    hT8 = np.ascontiguousarray(
        np.clip(np.asarray(inputs["hidden_states"][b].T, dtype=np.float32),
                -240.0, 240.0).astype(f8))
    return {
        "hT": hT, "hT8": hT8, "w_qf": w_qf, "w_kva": w_kva,
        "w_kbn": w_kbn, "w_kbr": w_kbr, "w_vb": w_vb, "w_ob": w_ob,
        "cos2": cos2.astype(bf16), "ssin2": ssin2.astype(bf16),
        "dmask": dmask.astype(bf16),
    }


_NC_CACHE = {}


def kernel(**inputs) -> np.ndarray:
    from concourse.bass_utils import run_bass_kernel_spmd

    C = CFG
    if "nc" not in _NC_CACHE:
        _NC_CACHE["nc"] = build_nc(C)
    nc = _NC_CACHE["nc"]

    in_maps = [host_inputs(C, inputs, c) for c in range(8)]
    res = run_bass_kernel_spmd(nc, in_maps, core_ids=list(range(8)))

    B = inputs["hidden_states"].shape[0]
    groups = 8 // B
    out = np.zeros((B, C.S, C.HID), dtype=np.float32)
    for c in range(8):
        out[c // groups] += res.results[c]["outp"]
    return out
